# revision 18
# baseline (speedup 1.0000x reference)
"""2-layer GCN (GCNConv 128->128->64, N=50000, E=800000) on 8 TRN2 NeuronCores.

Strategy v4 (dst-sharded, aggregate-first, host-built layer-1 table,
piece-pipelined AllGather with piece-aligned layer-2 gather windows):
  out = relu(A_hat @ (relu(A_hat @ x @ W1 + b1)) @ W2 + b2),  A_hat = D^-1/2 (A+I) D^-1/2
  - Layer-1 gather table x1' = Dis*x is built ON HOST (bf16, wrapped layout) and
    uploaded in full to every core: no first AllGather.
  - Dst-side Dis is applied once per 128-node block as a column scale on the
    aggregated PSUM tile; src-side Dis is folded into the tables. One-hot
    scatter matrices are iota==dst_rel built with tensor_scalar (DVE 4x mode).
  - The layer-2 table (Dis*relu(h1), bf16) is exchanged in NCC piece-contiguous
    AllGathers, each fired as soon as its block range finishes layer 1. The x2
    table is piece-major, and layer-2 edges are grouped by source PIECE, so
    gathers from already-landed pieces overlap the remaining AllGather pieces
    (DRAM deps are byte-range tracked).
  - Edges are grouped (dst-core, chunk of CH dst blocks, gather window, block),
    tiled in 128-edge tiles, gathered 8 tiles per dma_gather call (the ucode
    SWDGE ring caps one call at 1024 descriptors), round-robin over 4 queues.
  - All inputs ship as ONE packed buffer per core (fewer PJRT args = much
    cheaper per-exec dispatch on the axon path).
Host-side work is index prep + the x1' table build; output concat at the end.
"""

import numpy as np
import ml_dtypes

import concourse.bass as bass
import concourse.bacc as bacc
import concourse.mybir as mybir
import concourse.tile as tile
from concourse.bass_utils import run_bass_kernel_spmd
from concourse.library_config import mlp
from concourse.masks import make_identity

P = 128
N_NODES = 50000
N_EDGES = 800000
IN_CH = 128
HID_CH = 128
OUT_CH = 64
N_CORES = 8
NSH = N_NODES // N_CORES           # 6250 nodes per core
NBLK = (NSH + P - 1) // P          # 49 blocks per core (48 full + 106)
NFULL = NSH // P                   # 48
NTAIL = NSH - NFULL * P            # 106
NWRAP = NBLK * P                   # 6272
VLO = 32768                        # int16 gather window width
VTAB = N_CORES * P * NBLK          # 50176 table rows in [*, 128] view
CH = 4                             # dst blocks per gather chunk
CHUNKS = [list(range(c, min(c + CH, NBLK))) for c in range(0, NBLK, CH)]
import os as _os
# ucode's SWDGE ring is fixed at 1024 descriptors per call; >1024-idx gather
# calls crash NRT.
GCAP = int(_os.environ.get("K_GCAP", "8"))     # tiles (x128 idxs) per gather call
SHARED_AG = _os.environ.get("K_SHARED", "1") == "1"   # Shared-output AllGather
NCC = int(_os.environ.get("K_NCC", "3"))       # AllGather pieces (overlap w/ L1)
_AGB = _os.environ.get("K_AGBOUNDS", "")       # piece bounds in chunks, e.g. "6,10"

BF16 = mybir.dt.bfloat16
F32 = mybir.dt.float32
NPBF16 = ml_dtypes.bfloat16

LAST_RESULT = None  # for test harness: BassKernelResults of last run


def _chunks(t, cap=GCAP):
    """Split t tiles into balanced chunks of <= cap (16 -> 8+8, not 15+1)."""
    if t == 0:
        return []
    n = -(-t // cap)
    base, rem = divmod(t, n)
    return [base + (1 if i < rem else 0) for i in range(n)]


def _pieces():
    """AllGather piece plan: list of (b0, b1, end_chunk_idx)."""
    if _AGB:
        mids = [int(s) for s in _AGB.split(",") if s]
        bounds = [0] + mids + [len(CHUNKS)]
    else:
        bounds = [round(i * len(CHUNKS) / NCC) for i in range(NCC + 1)]
    out = []
    for p in range(len(bounds) - 1):
        c0, c1 = bounds[p], bounds[p + 1]
        if c0 == c1:
            continue
        out.append((CHUNKS[c0][0], CHUNKS[c1 - 1][-1] + 1, c1 - 1))
    return out


def _edge_plan(srow, dst, windows):
    """Slot plan for one layer given each edge's gather-table row `srow` and
    the gather windows (list of (w0, w1) covering [0, VTAB), width <= 32768).

    Slots are grouped (core, chunk, window, block) and 128-tile padded per
    (block, window) with shared (max-over-cores) tile counts.
    """
    NW = len(windows)
    wstarts = np.array([w0 for w0, _ in windows], np.int64)
    wends = np.array([w1 for _, w1 in windows], np.int64)
    assert all(w1 - w0 <= 32768 for w0, w1 in windows)
    win = np.searchsorted(wends, srow, side="right")
    assert (srow >= wstarts[win]).all() and (srow < wends[win]).all()

    core_of = dst // NSH
    io = dst - core_of * NSH
    blk = io // P
    drel = io % P

    cnt = np.bincount((core_of * NBLK + blk) * NW + win,
                      minlength=N_CORES * NBLK * NW).reshape(N_CORES, NBLK, NW)
    t_bw = np.ceil(cnt.max(axis=0) / P).astype(np.int64)   # [NBLK, NW]

    sect_order = []
    for blocks in CHUNKS:
        for w in range(NW):
            for b in blocks:
                sect_order.append((b, w))
    sect_tiles = np.array([t_bw[b, w] for b, w in sect_order], np.int64)
    sect_tile_base = np.concatenate([[0], np.cumsum(sect_tiles)])[:-1]
    T_total = int(sect_tiles.sum())
    EPC = T_total * P

    NSECT = len(sect_order)
    sect_index = np.zeros((NBLK, NW), np.int64)
    for s, (b, w) in enumerate(sect_order):
        sect_index[b, w] = s

    gkey = core_of * NSECT + sect_index[blk, win]
    order = np.argsort(gkey, kind="stable")
    g_sorted = gkey[order]
    gcnt = np.bincount(g_sorted, minlength=N_CORES * NSECT)
    group_start = np.concatenate([[0], np.cumsum(gcnt)])[:-1]
    pos_in_group = np.arange(len(srow)) - group_start[g_sorted]
    slot = sect_tile_base[g_sorted % NSECT] * P + pos_in_group
    edge_core = g_sorted // NSECT

    idx_rows = np.zeros((N_CORES, EPC), np.int64)
    dstrel = np.full((N_CORES, EPC), -1.0, np.float32)
    idx_rows[edge_core, slot] = (srow - wstarts[win])[order]
    dstrel[edge_core, slot] = drel[order].astype(np.float32)

    idxw = idx_rows.reshape(N_CORES, EPC // 16, 16).transpose(0, 2, 1)
    idxw = np.ascontiguousarray(np.tile(idxw, (1, 8, 1))).astype(np.int16)
    dstrelw = np.ascontiguousarray(
        dstrel.reshape(N_CORES, T_total, P).transpose(0, 2, 1)).astype(np.float32)
    return {"t_bw": t_bw, "T_total": T_total, "idxw": idxw,
            "dstrelw": dstrelw, "windows": windows}


def _host_prep(x, edge_index):
    """Index prep + host-built layer-1 gather table (bf16, wrapped layout)."""
    src = edge_index[0].astype(np.int64)
    dst = edge_index[1].astype(np.int64)
    deg = np.bincount(dst, minlength=N_NODES) + 1   # + self loop
    dis = (1.0 / np.sqrt(deg.astype(np.float64))).astype(np.float32)

    # layer-1 table: row q = k*128 + (i%128), block col r = i//128
    x1 = x * dis[:, None]
    v = np.arange(N_NODES)
    kk = v // NSH
    ii = v % NSH
    q = kk * P + (ii % P)
    r = ii // P
    x1tab = np.zeros((N_CORES * P, NBLK, IN_CH), np.float32)
    x1tab[q, r] = x1
    x1tab = np.ascontiguousarray(x1tab.reshape(N_CORES * P, NWRAP)).astype(NPBF16)

    sk = src // NSH
    si = src - sk * NSH
    sq = sk * P + (si % P)
    sr = si // P
    srow1 = sq * NBLK + sr

    # layer-2 table is piece-major: piece p holds blocks [b0, b1) as a
    # contiguous [1024, (b1-b0)*128] region
    pieces = _pieces()
    blk_piece_base = np.zeros(NBLK, np.int64)
    blk_piece_nb = np.zeros(NBLK, np.int64)
    blk_piece_b0 = np.zeros(NBLK, np.int64)
    rowbase = 0
    win2 = []
    for (b0, b1, _e) in pieces:
        nb = b1 - b0
        blk_piece_base[b0:b1] = rowbase
        blk_piece_nb[b0:b1] = nb
        blk_piece_b0[b0:b1] = b0
        win2.append((rowbase, rowbase + N_CORES * P * nb))
        rowbase += N_CORES * P * nb
    srow2 = (blk_piece_base[sr] + sq * blk_piece_nb[sr]
             + (sr - blk_piece_b0[sr]))
    if any(w1 - w0 > 32768 for w0, w1 in win2) \
            or _os.environ.get("K_PWIN", "1") != "1":
        # a piece exceeds the int16 window: fall back to fixed 32k windows
        win2 = [(0, VLO), (VLO, VTAB)]

    plan1 = _edge_plan(srow1, dst, [(0, VLO), (VLO, VTAB)])
    plan2 = _edge_plan(srow2, dst, win2)

    disw = np.zeros((N_CORES, P, NBLK), np.float32)
    nodes = np.arange(NBLK * P)
    valid = nodes < NSH
    for c in range(N_CORES):
        vv = np.zeros(NBLK * P, np.float32)
        vv[valid] = dis[c * NSH + nodes[valid]]
        disw[c] = vv.reshape(NBLK, P).T
    diswT = np.ascontiguousarray(np.broadcast_to(
        disw.transpose(0, 2, 1).reshape(N_CORES, 1, NWRAP), (N_CORES, P, NWRAP)))

    return {
        "x1tab": x1tab,
        "x1own": [np.ascontiguousarray(x1tab[c * P:(c + 1) * P])
                  for c in range(N_CORES)],
        "plan1": plan1, "plan2": plan2,
        "disw": disw, "diswT": diswT,
    }


def _pack_layout(T1, T2):
    """Byte layout of the single packed input buffer (512B-aligned segments)."""
    segs = {}
    off = 0

    def add(name, nbytes):
        nonlocal off
        segs[name] = off
        off += (nbytes + 511) // 512 * 512

    add("x1tab", N_CORES * P * NWRAP * 2)
    add("x1own", P * NWRAP * 2)
    add("idxw1", P * T1 * 8 * 2)
    add("idxw2", P * T2 * 8 * 2)
    add("dstrelw1", P * T1 * 4)
    add("dstrelw2", P * T2 * 4)
    add("disw", P * NBLK * 4)
    add("diswT", P * NWRAP * 4)
    add("w1", IN_CH * HID_CH * 4)
    add("b1", HID_CH * 4)
    add("w2", HID_CH * OUT_CH * 4)
    add("b2", OUT_CH * 4)
    segs["_total"] = (off + 511) // 512 * 512
    return segs


def _make_in_maps(prep, W1, b1, W2, b2):
    p1, p2 = prep["plan1"], prep["plan2"]
    segs = _pack_layout(p1["T_total"], p2["T_total"])
    NB = segs["_total"]
    maps = []
    for c in range(N_CORES):
        pk = np.zeros(NB, np.uint8)

        def put(name, arr):
            b = np.ascontiguousarray(arr).view(np.uint8).reshape(-1)
            pk[segs[name]:segs[name] + b.size] = b

        put("x1tab", prep["x1tab"])
        put("x1own", prep["x1own"][c])
        put("idxw1", p1["idxw"][c])
        put("idxw2", p2["idxw"][c])
        put("dstrelw1", p1["dstrelw"][c])
        put("dstrelw2", p2["dstrelw"][c])
        put("disw", prep["disw"][c])
        put("diswT", prep["diswT"][c])
        put("w1", np.asarray(W1, np.float32))
        put("b1", np.asarray(b1, np.float32).reshape(HID_CH, 1))
        put("w2", np.asarray(W2, np.float32))
        put("b2", np.asarray(b2, np.float32).reshape(OUT_CH, 1))
        maps.append({"pack": pk.reshape(1, NB)})
    return maps


def _build(prep):
    plan1, plan2 = prep["plan1"], prep["plan2"]
    T1, T2 = plan1["T_total"], plan2["T_total"]
    pieces = _pieces()

    nc = bacc.Bacc("TRN2", target_bir_lowering=False, num_devices=N_CORES,
                   num_swdge_queues=4)

    segs = _pack_layout(T1, T2)
    t_pack = nc.dram_tensor("pack", [1, segs["_total"]], mybir.dt.uint8,
                            kind="ExternalInput")
    t_out = nc.dram_tensor("out", [NSH, OUT_CH], F32, kind="ExternalOutput")

    def pview(name, dt, cols):
        """[128, cols] DMA view of a packed segment."""
        nb = P * cols * mybir.dt.size(dt)
        return t_pack.ap()[0:1, segs[name]:segs[name] + nb].bitcast(dt) \
            .rearrange("o (p x) -> (o p) x", x=cols)

    # piece-major flat buffers so each AllGather is contiguous
    x2_shard = nc.dram_tensor("x2_shard", [1, P * NWRAP], BF16)
    x2_full = nc.dram_tensor("x2_full", [1, N_CORES * P * NWRAP], BF16,
                             addr_space="Shared" if SHARED_AG else "Local")

    rg = [list(range(N_CORES))]
    gq = [0]  # round-robin gather queue counter

    with tile.TileContext(nc) as tc:
        with (
            tc.tile_pool(name="const", bufs=1) as cp,
            tc.tile_pool(name="sbuf", bufs=3) as sb,
            tc.tile_pool(name="gpool", bufs=2) as gp,
            tc.tile_pool(name="opool", bufs=3) as op,
            tc.tile_pool(name="psum", bufs=2, space="PSUM") as ps,
        ):
            nc.gpsimd.load_library(mlp)

            idx_sb1 = cp.tile([P, T1 * 8], mybir.dt.int16)
            nc.sync.dma_start(out=idx_sb1[:], in_=pview("idxw1", mybir.dt.int16, T1 * 8))
            dstrel_sb1 = cp.tile([P, T1], F32)
            nc.sync.dma_start(out=dstrel_sb1[:], in_=pview("dstrelw1", F32, T1))
            idx_sb2 = cp.tile([P, T2 * 8], mybir.dt.int16)
            nc.sync.dma_start(out=idx_sb2[:], in_=pview("idxw2", mybir.dt.int16, T2 * 8))
            dstrel_sb2 = cp.tile([P, T2], F32)
            nc.sync.dma_start(out=dstrel_sb2[:], in_=pview("dstrelw2", F32, T2))
            disw_sb = cp.tile([P, NBLK], F32)
            nc.sync.dma_start(out=disw_sb[:], in_=pview("disw", F32, NBLK))
            diswT_sb = cp.tile([P, NWRAP], F32)
            nc.sync.dma_start(out=diswT_sb[:], in_=pview("diswT", F32, NWRAP))

            iota_i = cp.tile([P, P], mybir.dt.int32)
            nc.gpsimd.iota(iota_i[:], pattern=[[1, P]], base=0, channel_multiplier=0)
            iota_bf = cp.tile([P, P], BF16)
            nc.vector.tensor_copy(out=iota_bf[:], in_=iota_i[:])

            ident_bf = cp.tile([P, P], BF16)
            make_identity(nc, ident_bf[:])
            ident_f = cp.tile([OUT_CH, OUT_CH], F32)
            make_identity(nc, ident_f[:])

            w1_f = cp.tile([IN_CH, HID_CH], F32)
            nc.sync.dma_start(out=w1_f[:], in_=pview("w1", F32, HID_CH))
            w1_bf = cp.tile([IN_CH, HID_CH], BF16)
            nc.vector.tensor_copy(out=w1_bf[:], in_=w1_f[:])
            w2_f = cp.tile([HID_CH, OUT_CH], F32)
            nc.sync.dma_start(out=w2_f[:], in_=pview("w2", F32, OUT_CH))
            w2_bf = cp.tile([HID_CH, OUT_CH], BF16)
            nc.vector.tensor_copy(out=w2_bf[:], in_=w2_f[:])
            b1_sb = cp.tile([HID_CH, 1], F32)
            nc.sync.dma_start(out=b1_sb[:], in_=pview("b1", F32, 1))
            b2_sb = cp.tile([OUT_CH, 1], F32)
            nc.sync.dma_start(
                out=b2_sb[:],
                in_=t_pack.ap()[0:1, segs["b2"]:segs["b2"] + OUT_CH * 4]
                .bitcast(F32).rearrange("o (p x) -> (o p) x", x=1))

            stage1 = cp.tile([P, NBLK, IN_CH], BF16)
            nc.sync.dma_start(
                out=stage1[:].rearrange("p b c -> p (b c)"),
                in_=pview("x1own", BF16, NWRAP))
            x2stage = cp.tile([P, NBLK, HID_CH], BF16)

            def layer(tabv, plan, idx_sb, dstrel_sb, stage, w_bf, b_sb, oc,
                      epilogue, after_chunk=None):
                t_bw = plan["t_bw"]
                windows = plan["windows"]
                NW = len(windows)
                Tg = 0
                for ci, blocks in enumerate(CHUNKS):
                    tw = [[int(t_bw[b, w]) for b in blocks] for w in range(NW)]
                    n_w = [sum(tw[w]) for w in range(NW)]
                    Tc = sum(n_w)
                    g = gp.tile([P, Tc, HID_CH], BF16, tag="g")
                    off = 0
                    for w, (w0, w1r) in enumerate(windows):
                        for n in _chunks(n_w[w]):
                            nc.gpsimd.dma_gather(
                                out_ap=g[:, off:off + n, :], in_ap=tabv[w0:w1r, :],
                                idxs_ap=idx_sb[:, 8 * (Tg + off): 8 * (Tg + off + n)],
                                num_idxs=n * P, num_idxs_reg=n * P,
                                elem_size=HID_CH, queue_num=gq[0] % 4,
                            )
                            gq[0] += 1
                            off += n

                    woff = np.concatenate([[0], np.cumsum(n_w)])[:-1]
                    boff = [0] * NW
                    for bi, b in enumerate(blocks):
                        tb = sum(tw[w][bi] for w in range(NW))
                        gidx = []
                        for w in range(NW):
                            a = int(woff[w]) + boff[w]
                            gidx += list(range(a, a + tw[w][bi]))
                            boff[w] += tw[w][bi]
                        ob = op.tile([P, tb, P], BF16, tag="ob")
                        # one-hots: per-partition scalar compare (DVE 4x mode)
                        for jj, j in enumerate(gidx):
                            nc.vector.tensor_scalar(
                                out=ob[:, jj, :], in0=iota_bf[:],
                                scalar1=dstrel_sb[:, Tg + j: Tg + j + 1],
                                scalar2=None,
                                op0=mybir.AluOpType.is_equal,
                            )
                        tps = ps.tile([HID_CH, P], F32, tag="tps")
                        for jj, j in enumerate(gidx):
                            nc.tensor.matmul(
                                out=tps[:], lhsT=g[:, j, :], rhs=ob[:, jj, :],
                                start=(jj == 0), stop=False,
                            )
                        nc.tensor.matmul(
                            out=tps[:], lhsT=stage[:, b, :], rhs=ident_bf[:],
                            start=(tb == 0), stop=True,
                        )
                        # dst-side Dis as a column scale during PSUM -> SBUF
                        t_sb = sb.tile([HID_CH, P], BF16, tag="tsb")
                        nc.vector.tensor_tensor(
                            out=t_sb[:], in0=tps[:],
                            in1=diswT_sb[:, b * P:(b + 1) * P],
                            op=mybir.AluOpType.mult,
                        )
                        ups = ps.tile([oc, P], F32, tag="ups")
                        nc.tensor.matmul(out=ups[:], lhsT=w_bf[:], rhs=t_sb[:],
                                         start=True, stop=True)
                        epilogue(b, ups)
                    Tg += Tc
                    if after_chunk is not None:
                        after_chunk(ci)

            # ---- layer 1 (+ pipelined AllGather pieces) ----
            def epi1(b, ups):
                h1t = sb.tile([HID_CH, P], BF16, tag="h1t")
                nc.scalar.activation(out=h1t[:], in_=ups[:],
                                     func=mybir.ActivationFunctionType.Relu,
                                     bias=b1_sb[:, :1])
                trp = ps.tile([P, HID_CH], BF16, tag="trp")
                nc.tensor.transpose(out=trp[:], in_=h1t[:], identity=ident_bf[:])
                nc.vector.tensor_tensor(
                    out=x2stage[:, b, :], in0=trp[:],
                    in1=disw_sb[:, b:b + 1].to_broadcast([P, HID_CH]),
                    op=mybir.AluOpType.mult,
                )

            piece_at_chunk = {e: (b0, b1) for (b0, b1, e) in pieces}
            piece_elem_base = {}
            acc = 0
            for (b0, b1, e) in pieces:
                piece_elem_base[e] = acc
                acc += (b1 - b0) * HID_CH

            def ag_piece(ci):
                if ci not in piece_at_chunk:
                    return
                b0, b1 = piece_at_chunk[ci]
                span = (b1 - b0) * HID_CH
                base = piece_elem_base[ci]
                shv = x2_shard.ap()[0:1, P * base: P * (base + span)] \
                    .rearrange("o (p s) -> (o p) s", s=span)
                fuv = x2_full.ap()[0:1, N_CORES * P * base:
                                   N_CORES * P * (base + span)] \
                    .rearrange("o (q s) -> (o q) s", s=span)
                nc.sync.dma_start(
                    out=shv,
                    in_=x2stage[:, b0:b1, :].rearrange("p b c -> p (b c)"))
                nc.gpsimd.collective_compute(
                    "AllGather", mybir.AluOpType.bypass, replica_groups=rg,
                    ins=[shv.opt()], outs=[fuv.opt()],
                )

            tabv1 = t_pack.ap()[0:1, segs["x1tab"]:
                                segs["x1tab"] + N_CORES * P * NWRAP * 2] \
                .bitcast(BF16).rearrange("o (v c) -> (o v) c", c=HID_CH)
            layer(tabv1, plan1, idx_sb1, dstrel_sb1, stage1, w1_bf, b1_sb,
                  HID_CH, epi1, after_chunk=ag_piece)

            # ---- layer 2 ----
            def epi2(b, ups):
                nb = P if b < NFULL else NTAIL
                h2t = sb.tile([OUT_CH, P], F32, tag="h2t")
                nc.scalar.activation(out=h2t[:], in_=ups[:],
                                     func=mybir.ActivationFunctionType.Relu,
                                     bias=b2_sb[:, :1])
                trp2 = ps.tile([P, OUT_CH], F32, tag="trp2")
                nc.tensor.transpose(out=trp2[:], in_=h2t[:], identity=ident_f[:])
                outt = sb.tile([P, OUT_CH], F32, tag="outt")
                nc.vector.tensor_copy(out=outt[:], in_=trp2[:])
                nc.sync.dma_start(out=t_out[b * P: b * P + nb, :], in_=outt[:nb, :])

            tabv2 = x2_full.ap().rearrange("o (v c) -> (o v) c", c=HID_CH)
            layer(tabv2, plan2, idx_sb2, dstrel_sb2, x2stage, w2_bf, b2_sb,
                  OUT_CH, epi2)

    nc.compile()
    return nc


def kernel(x, edge_index, W1, b1, W2, b2, _trace=False):
    global LAST_RESULT
    x = np.asarray(x, dtype=np.float32)
    edge_index = np.asarray(edge_index, dtype=np.int32)

    prep = _host_prep(x, edge_index)
    nc = _build(prep)
    in_maps = _make_in_maps(prep, W1, b1, W2, b2)

    res = run_bass_kernel_spmd(nc, in_maps, core_ids=list(range(N_CORES)),
                               trace=_trace)
    LAST_RESULT = res
    out = np.concatenate([res.results[k]["out"] for k in range(N_CORES)], axis=0)
    return out.astype(np.float32)
